# revision 13
# baseline (speedup 1.0000x reference)
"""Causal self-attention (B=2, L=2048, E=2048, H=16, D=128) on 8 trn2 cores.

Sharding: Megatron-style tensor parallel over heads (2 heads/core) with
minimal host<->device traffic:
  - x is sharded: core r receives xT slice [E, 512] for (b=r//4, l-block r%4);
    an on-device AllGather rebuilds the full activation in device DRAM.
  - RoPE sin/cos tables are sharded 1/8 per core and AllGathered on device.
  - Causal masks are built on device with affine_select (no wire traffic).
  - Each core computes a full [B*L, E] partial of the output projection; an
    on-device ReduceScatter sums partials so each core returns only its
    [512, E] slice.  Host concatenates slices and adds the bias.

Device kernel per core, per batch:
  - qT/kT [D=128, L] for both heads via weight-stationary matmuls streaming
    the AllGathered xT once per batch (host pre-permutes Wq/Wk columns to
    (evens, odds) so RoPE is a handful of wide DVE ops).
  - attention works on transposed scores: sT[k, q] = kT.T @ qT needs no
    DMA-transposes at all; exp(sT) chunks feed attn@V (N=512 matmuls) while
    DVE accumulates the chunk sum, from which one ones-vector matmul per
    q-block produces softmax denominators; the normalization multiplies
    attn@V output by a PE-broadcast reciprocal.
  - out-projection contracts over D with per-head stationary tiles, writes
    bf16 partials to DRAM for the ReduceScatter.

All inputs arrive packed in a single [6272, 512] bf16 tensor per core — the
axon execute path pays ~1 ms per IO buffer per call, so buffer count is 3
(packed input, output-init, output).
"""

import os

import numpy as np
import ml_dtypes

import concourse.bass as bass
import concourse.tile as tile
from concourse import bacc, mybir
from concourse.bass_utils import run_bass_kernel_spmd

BF16 = mybir.dt.bfloat16
F32 = mybir.dt.float32
AF = mybir.ActivationFunctionType
ALU = mybir.AluOpType

B, L, E = 2, 2048, 2048
H, D = 16, 128
NCORES = 8
HPC = H // NCORES          # heads per core
KT = E // 128              # 16 contraction tiles
LC = L // 512              # 4 column chunks of L per batch
QT = L // 128              # 16 q tiles
QB = L // 512              # 4 q blocks of 512
THETA = 10000.0
NEG = -1.0e30

_PROG = None


def _build_program():
    nc = bacc.Bacc("TRN2", target_bir_lowering=False, debug=False,
                   enable_asserts=False, num_devices=NCORES)

    # single packed input: rows [0:2048] xT slice, [2048:2176] rope-table
    # shard, [2176:5248] qkv weights ([p, kt, h, j, d] order), [5248:6272]
    # out-proj weights ([p, h, e] order)
    pk_d = nc.dram_tensor("pk", [6272, 512], BF16, kind="ExternalInput").ap()
    y_d = nc.dram_tensor("y", [B * L // NCORES, E], BF16, kind="ExternalOutput").ap()

    grp = [list(range(NCORES))]

    with tile.TileContext(nc) as tc:
        with tc.tile_pool(name="consts", bufs=1) as cpool, \
             tc.tile_pool(name="xt", bufs=8) as xpool, \
             tc.tile_pool(name="rope", bufs=6) as rpool, \
             tc.tile_pool(name="qkv", bufs=4) as qkvpool, \
             tc.tile_pool(name="pp", bufs=8) as ppool, \
             tc.tile_pool(name="small", bufs=8) as spool, \
             tc.tile_pool(name="pacc", bufs=4) as papool, \
             tc.tile_pool(name="rbs", bufs=2) as rbpool, \
             tc.tile_pool(name="ys", bufs=2) as ypool, \
             tc.tile_pool(name="ps", bufs=8, space="PSUM") as pspool, \
             tc.tile_pool(name="dram", bufs=1, space="DRAM") as dram:

            # ---- weights ----
            w_sb = cpool.tile([128, KT, HPC, 3, 128], BF16, tag="w")
            nc.sync.dma_start(
                w_sb[:], pk_d[2176:5248, :].rearrange("(p r) c -> p (r c)", p=128))
            wo_sb = cpool.tile([128, HPC, E], BF16, tag="wo")
            nc.sync.dma_start(
                wo_sb[:], pk_d[5248:6272, :].rearrange("(p r) c -> p (r c)", p=128))


            # ---- causal masks for the 4 diagonal chunk offsets ----
            masks = []
            for d in range(4):
                m = cpool.tile([128, 512], F32, tag=f"mask{d}")
                nc.gpsimd.memset(m[:], 0.0)
                # keep 0 where (col - part - 128*d) >= 0, else -1e30
                nc.gpsimd.affine_select(
                    m[:], m[:], pattern=[[1, 512]], compare_op=ALU.is_ge,
                    fill=NEG, base=-128 * d, channel_multiplier=-1)
                masks.append(m)

            ones_col = cpool.tile([128, 1], BF16, tag="ones_col")
            nc.gpsimd.memset(ones_col[:], 1.0)
            ones_row = cpool.tile([1, 128], BF16, tag="ones_row")
            nc.gpsimd.memset(ones_row[:], 1.0)

            # ---- RoPE tables: AllGather 1/8 shards, convert to f32 ----
            tb_b = dram.tile([32, L], BF16, tag="tb_b")
            tblg = dram.tile([32 * NCORES, L], BF16, tag="tblg")
            nc.sync.dma_start(tb_b[:], pk_d[2048:2176, :])
            nc.gpsimd.collective_compute(
                "AllGather", ALU.bypass, replica_groups=grp,
                ins=[tb_b.opt()], outs=[tblg.opt()])
            cs = cpool.tile([128, L], F32, tag="cs")
            ss = cpool.tile([128, L], F32, tag="ss")
            for dst, r0 in ((cs, 0), (ss, 128)):
                for c4 in range(4):
                    tmp = xpool.tile([128, 512], BF16, tag="xt",
                                     name=f"tbl_{r0}_{c4}")
                    nc.sync.dma_start(tmp[:], tblg[r0:r0 + 128,
                                                   c4 * 512:(c4 + 1) * 512])
                    nc.vector.tensor_copy(dst[:, c4 * 512:(c4 + 1) * 512], tmp[:])

            for rep in range(int(os.environ.get("KREP", "1"))):
                # ---- AllGather x shards ----
                xs_b = dram.tile([E, 512], BF16, tag="xs_b")
                xg = dram.tile([NCORES * E, 512], BF16, tag="xg")
                nc.sync.dma_start(xs_b[:], pk_d[0:E, :])
                nc.gpsimd.collective_compute(
                    "AllGather", ALU.bypass, replica_groups=grp,
                    ins=[xs_b.opt()], outs=[xg.opt()])

                ypart = dram.tile([B * L, E], BF16, tag="ypart")

                for b in range(B):
                    # ---- QKV projection + RoPE for both heads ----
                    qT = [qkvpool.tile([128, L], BF16, tag="qT",
                                       name=f"qT_b{b}h{h}") for h in range(HPC)]
                    kT = [qkvpool.tile([128, L], BF16, tag="kT",
                                       name=f"kT_b{b}h{h}") for h in range(HPC)]
                    vTs = [qkvpool.tile([128, L], BF16, tag="vTs",
                                        name=f"vTs_b{b}h{h}") for h in range(HPC)]
                    for lc in range(LC):
                        ls = lc * 512
                        pA = [pspool.tile([128, 512], F32, tag="ps",
                                          name=f"pA{b}_{lc}_{h}") for h in range(HPC)]
                        pB = [pspool.tile([128, 512], F32, tag="ps",
                                          name=f"pB{b}_{lc}_{h}") for h in range(HPC)]
                        pV = [pspool.tile([128, 512], F32, tag="ps",
                                          name=f"pV{b}_{lc}_{h}") for h in range(HPC)]
                        for kt in range(KT):
                            xt = xpool.tile([128, 512], BF16, tag="xt")
                            row0 = (b * LC + lc) * E + kt * 128
                            nc.sync.dma_start(xt[:], xg[row0:row0 + 128, :])
                            st = kt == 0
                            sp = kt == KT - 1
                            for h in range(HPC):
                                nc.tensor.matmul(pA[h][:], w_sb[:, kt, h, 0, :], xt[:],
                                                 start=st, stop=sp)
                                nc.tensor.matmul(pB[h][:], w_sb[:, kt, h, 1, :], xt[:],
                                                 start=st, stop=sp)
                                nc.tensor.matmul(pV[h][:], w_sb[:, kt, h, 2, :], xt[:],
                                                 start=st, stop=sp)
                        for h in range(HPC):
                            # RoPE: rows of A/B are [q-even|k-even] / [q-odd|k-odd]
                            t1 = rpool.tile([128, 512], F32, tag="rt")
                            nc.vector.tensor_mul(t1[:], pA[h][:], cs[:, ls:ls + 512])
                            t2 = rpool.tile([128, 512], F32, tag="rt")
                            nc.vector.tensor_mul(t2[:], pB[h][:], ss[:, ls:ls + 512])
                            t3 = rpool.tile([128, 512], F32, tag="rt")
                            nc.vector.tensor_mul(t3[:], pA[h][:], ss[:, ls:ls + 512])
                            t4 = rpool.tile([128, 512], F32, tag="rt")
                            nc.vector.tensor_mul(t4[:], pB[h][:], cs[:, ls:ls + 512])
                            nc.vector.tensor_sub(qT[h][0:64, ls:ls + 512],
                                                 t1[0:64, :], t2[0:64, :])
                            nc.vector.tensor_sub(kT[h][0:64, ls:ls + 512],
                                                 t1[64:128, :], t2[64:128, :])
                            nc.vector.tensor_add(qT[h][64:128, ls:ls + 512],
                                                 t3[0:64, :], t4[0:64, :])
                            nc.vector.tensor_add(kT[h][64:128, ls:ls + 512],
                                                 t3[64:128, :], t4[64:128, :])
                            nc.scalar.copy(vTs[h][:, ls:ls + 512], pV[h][:])

                    vN = [qkvpool.tile([128, KT, 128], BF16, tag="vN",
                                       name=f"vN_b{b}h{h}") for h in range(HPC)]
                    for h in range(HPC):
                        nc.scalar.dma_start_transpose(out=vN[h][:], in_=vTs[h][:])

                    # ---- attention (transposed scores; no P transposes) ----
                    outT = [qkvpool.tile([128, L], BF16, tag="oT",
                                         name=f"oT_b{b}h{h}") for h in range(HPC)]
                    for h in range(HPC):
                        pvs, paccs = [], []
                        for qb in range(QB):
                            qs = qb * 512
                            nch = 4 * (qb + 1)
                            pv = pspool.tile([128, 512], F32, tag="ps",
                                             name=f"pv{b}_{h}_{qb}")
                            pacc = papool.tile([128, 512], BF16, tag="pacc",
                                              name=f"pacc{b}_{h}_{qb}")
                            for kb in range(nch):
                                s = pspool.tile([128, 512], F32, tag="ps")
                                nc.tensor.matmul(
                                    s[:], kT[h][:, kb * 128:(kb + 1) * 128],
                                    qT[h][:, qs:qs + 512], start=True, stop=True)
                                dd = kb - 4 * qb
                                if dd >= 0:
                                    nc.vector.tensor_tensor(
                                        s[:], s[:], masks[dd][:], op=ALU.add)
                                pt = ppool.tile([128, 512], BF16, tag="pt")
                                nc.scalar.activation(pt[:], s[:], AF.Exp)
                                if kb == 0:
                                    nc.vector.tensor_copy(pacc[:], pt[:])
                                else:
                                    nc.vector.tensor_add(pacc[:], pacc[:], pt[:])
                                nc.tensor.matmul(pv[:], vN[h][:, kb, :], pt[:],
                                                 start=(kb == 0),
                                                 stop=(kb == nch - 1))
                            pvs.append(pv)
                            paccs.append(pacc)
                        for qb in range(QB):
                            qs = qb * 512
                            dn = pspool.tile([1, 512], F32, tag="ps")
                            nc.tensor.matmul(dn[:], ones_col[:], paccs[qb][:],
                                             start=True, stop=True)
                            rinv = spool.tile([1, 512], F32, tag="ri")
                            nc.vector.reciprocal(rinv[:], dn[:])
                            rinv_h = spool.tile([1, 512], BF16, tag="rih")
                            nc.vector.tensor_copy(rinv_h[:], rinv[:])
                            rb = pspool.tile([128, 512], F32, tag="ps")
                            nc.tensor.matmul(rb[:], ones_row[:], rinv_h[:],
                                             start=True, stop=True)
                            rbs = rbpool.tile([128, 512], F32, tag="rbs")
                            nc.scalar.copy(rbs[:], rb[:])
                            nc.vector.tensor_mul(outT[h][:, qs:qs + 512],
                                                 pvs[qb][:], rbs[:])

                    # ---- output projection (partial over this core's heads) ----
                    for qt in range(QT):
                        ysb = ypool.tile([128, E], BF16, tag="ysb")
                        qs = qt * 128
                        yp = [pspool.tile([128, 512], F32, tag="ps",
                                          name=f"yp{b}_{qt}_{ec}") for ec in range(4)]
                        for h in range(HPC):
                            for ec in range(4):
                                nc.tensor.matmul(
                                    yp[ec][:], outT[h][:, qs:qs + 128],
                                    wo_sb[:, h, ec * 512:(ec + 1) * 512],
                                    start=(h == 0), stop=(h == HPC - 1))
                        for ec in range(4):
                            es = ec * 512
                            if ec % 2 == 0:
                                nc.scalar.copy(ysb[:, es:es + 512], yp[ec][:])
                            else:
                                nc.vector.tensor_copy(ysb[:, es:es + 512], yp[ec][:])
                        nc.sync.dma_start(
                            ypart[b * L + qs:b * L + qs + 128, :], ysb[:])

                # ---- ReduceScatter partials; write this core's slice ----
                ysl = dram.tile([B * L // NCORES, E], BF16, tag="ysl")
                nc.gpsimd.collective_compute(
                    "ReduceScatter", ALU.add, replica_groups=grp,
                    ins=[ypart.opt()], outs=[ysl.opt()])
                nc.sync.dma_start(y_d[:], ysl[:])

    nc.compile()
    return nc


def _get_program():
    global _PROG
    if _PROG is None:
        _PROG = _build_program()
    return _PROG


def make_in_maps(x, Wq, Wk, Wv, Wo):
    """Host-side sharding/layout prep. Returns list of 8 per-core input maps."""
    bf = ml_dtypes.bfloat16
    x = np.asarray(x, np.float32)
    Wq = np.asarray(Wq, np.float32)
    Wk = np.asarray(Wk, np.float32)
    Wv = np.asarray(Wv, np.float32)
    Wo = np.asarray(Wo, np.float32)

    xT = np.ascontiguousarray(x.transpose(0, 2, 1)).astype(bf)  # [B, E, L]

    inv = THETA ** (-np.arange(0, D, 2, dtype=np.float32) / D)  # [64]
    ang = np.arange(L, dtype=np.float32)[:, None] * inv[None, :]  # [L, 64]
    cosf = np.concatenate([np.cos(ang).T] * 2, axis=0)  # [128, L]
    sinf = np.concatenate([np.sin(ang).T] * 2, axis=0)
    tbl = np.ascontiguousarray(
        np.concatenate([cosf, sinf], axis=0)).astype(bf)  # [256, L]

    qscale = np.float32(D ** -0.5)
    ev = np.arange(0, D, 2)
    od = np.arange(1, D, 2)

    maps = []
    for core in range(NCORES):
        w_all = np.empty((E, HPC, 3, 128), np.float32)
        for h in range(HPC):
            g = core * HPC + h
            c0 = g * D
            w_all[:, h, 0, 0:64] = Wq[:, c0 + ev] * qscale
            w_all[:, h, 0, 64:128] = Wk[:, c0 + ev]
            w_all[:, h, 1, 0:64] = Wq[:, c0 + od] * qscale
            w_all[:, h, 1, 64:128] = Wk[:, c0 + od]
            w_all[:, h, 2, :] = Wv[:, c0:c0 + D]
        wo_c = Wo[core * HPC * D:(core + 1) * HPC * D, :]
        b_r, l_r = core // LC, (core % LC) * 512
        pk = np.empty((6272, 512), bf)
        pk[0:2048] = xT[b_r, :, l_r:l_r + 512]
        pk[2048:2176] = tbl[core * 32:(core + 1) * 32, :].reshape(128, 512)
        pk[2176:5248] = (w_all.reshape(KT, 128, HPC, 3, 128)
                         .transpose(1, 0, 2, 3, 4).reshape(3072, 512).astype(bf))
        pk[5248:6272] = (wo_c.reshape(HPC, 128, E)
                         .transpose(1, 0, 2).reshape(1024, 512).astype(bf))
        maps.append({"pk": pk})
    return maps


def kernel(x, Wq, Wk, Wv, Wo, bo):
    nc = _get_program()
    maps = make_in_maps(x, Wq, Wk, Wv, Wo)
    res = run_bass_kernel_spmd(nc, maps, core_ids=list(range(NCORES)))
    y = np.concatenate(
        [np.asarray(res.results[c]["y"], np.float32) for c in range(NCORES)],
        axis=0).reshape(B, L, E)
    y += np.asarray(bo, np.float32)[None, None, :]
    return y.astype(np.float32)
